# revision 2
# baseline (speedup 1.0000x reference)
"""Trainium2 Bass kernel for the CrossAttention reference module.

  claim = x[claim_index]; evidence = x[evidence_index]
  wc = claim @ Wc + bc; we = evidence @ We + be
  S = wc @ we.T + blockdiag_mask(batch[claim_index], batch[evidence_index])
  A = softmax(S, -1); cn = A @ evidence
  a = concat([claim, cn, claim-cn, claim*cn]) @ Wa + ba
  out = segment_mean(a, batch[claim_index], 64)

Sharding (per the hint: devices hold gathered claim/evidence rows):
claims are sorted by graph and split 512 per core across 8 cores.
Evidence is sorted by graph on the host, so each core's evidence set is a
CONTIGUOUS slice of the sorted-evidence matrix (max span 1236 rows for
these inputs -> NE_LOC rounded to 1280).  The host pre-gathers x rows into
bf16 matrices tiled in the exact SBUF layouts the kernel wants (row-major
and transposed), so the device does plain contiguous DMAs: no gathers, no
on-device transposes, no dtype casts.

Device-side math (per core), all matmuls bf16 with fp32 PSUM accumulation:
  we_p^T = We^T @ evT + be            [64, NE_LOC]   (+ 64 one-hot rows from host)
  wc_p^T = Wc^T @ clT + bc            [64, 512]      (+ 64 one-hot rows)
  S^T[e-tile] = we_aug[:,e128]^T . wc_aug   -> [128e, 512c] per tile
  P^T = exp(S^T - 1074)               (one-hot trick adds +1024 on same-graph
                                       pairs; exp underflows to exactly 0 for
                                       cross-graph pairs, and the -50 shift is
                                       row-constant so softmax ignores it)
  cn^T[h] += ev[e,h128]^T . P^T       (PV with NO transposes: evidence rows are
  rowsum  += ones^T . P^T              the contraction dim in S^T space)
  cn^T *= 1/rowsum (broadcast via ones-matmul)
  a = clT^T.W1' + cnT^T.W2' + (clT*cnT)^T.W3'   (Wa folded on host:
        W1'=Wa0+Wa2, W2'=Wa1-Wa2, W3'=Wa3 -- the claim-cn concat block folds away)
  seg = onehot(graph)^T . a           (segment sums; host divides by counts
                                       and adds ba: mean(a+ba)=mean(a)+ba)
"""

import sys

if "/opt/trn_rl_repo" not in sys.path:
    sys.path.insert(0, "/opt/trn_rl_repo")

import ml_dtypes
import numpy as np

import concourse.bass as bass
import concourse.mybir as mybir
import concourse.tile as tile
from concourse.bass_utils import run_bass_kernel_spmd
from concourse.vector_clock import ScopedClock

P = 128
NHID = 512
PROJ = 64
NC_ALL = 4096
NE = 8192
NG = 64
N_CORES = 8
NC_LOC = NC_ALL // N_CORES  # 512 claims per core
KT = NHID // P              # 4 hidden k-tiles
CT = NC_LOC // P            # 4 claim tiles per core
KO = 12                     # folded concat k-tiles (3 blocks x 4)
MAG = 32.0                  # sqrt(1024): one-hot scale
EXP_BIAS = -(MAG * MAG + 50.0)  # exp(S + 1024 - 1074) = exp(S - 50)

f32 = mybir.dt.float32
bf16 = mybir.dt.bfloat16
AF = mybir.ActivationFunctionType
ALU = mybir.AluOpType
nbf16 = ml_dtypes.bfloat16


class _PatchedTileContext(tile.TileContext):
    """Workaround: this neuronxcc/walrus build rejects InstDrain carrying
    sync waits ("Too many sync wait commands").  Collect the final drain's
    waits on nops (one wait each) and emit the drain itself wait-free."""

    def _drain_and_barrier(self, tick_clock, wait_clock):
        nc = self.nc
        nop0 = nc.sync.nop(nofuse=True)
        wait_clock.add_sem_waits(nop0.ins, ScopedClock({None: tick_clock.global_clock}))
        si = nop0.ins.sync_info
        waits = list(si.on_wait) if si and si.on_wait else []
        if si and len(waits) > 1:
            del si.on_wait[1:]
            for w in waits[1:]:
                extra = nc.sync.nop(nofuse=True)
                if extra.ins.sync_info is None:
                    extra.ins.sync_info = mybir.SyncInfo(on_wait=[w], on_update=[])
                else:
                    extra.ins.sync_info.on_wait.append(w)
        drain_inst = nc.sync.drain()
        wait_clock.add_sem_waits(
            drain_inst.ins, ScopedClock({None: tick_clock.global_clock})
        )
        dsi = drain_inst.ins.sync_info
        if dsi and dsi.on_wait:
            del dsi.on_wait[:]
        nc.all_engine_barrier()
        popped = nc._tile_sem_poison_stack.pop()
        assert popped is self._sem_poison
        nc.clear_and_free_semaphores(list(self.sems.allocated().values()))
        nc.all_engine_barrier()


def _split_excess_waits(nc: bass.Bass, limit: int = 1) -> None:
    """This walrus build rejects instructions carrying more than ~1 sync
    wait.  Move excess waits onto injected same-engine nops (engines are
    in-order, so gating a preceding nop gates the instruction)."""
    for f in nc.m.functions:
        for bb in f.blocks:
            new_insts = []
            for inst in bb.instructions:
                si = getattr(inst, "sync_info", None)
                if si is not None and si.on_wait and len(si.on_wait) > limit:
                    keep = list(si.on_wait[-limit:])
                    excess = list(si.on_wait[:-limit])
                    for w in excess:
                        nop = mybir.InstNoOp(
                            name=f"I-{nc.next_id()}", engine=inst.engine,
                            ins=[], outs=[],
                            sync_info=mybir.SyncInfo(on_wait=[w], on_update=[]))
                        new_insts.append(nop)
                    del si.on_wait[:]
                    si.on_wait.extend(keep)
                new_insts.append(inst)
            bb.instructions[:] = new_insts


def build_nc(ne_loc: int = 1280, reps: int = 1) -> bass.Bass:
    assert ne_loc % P == 0
    ET = ne_loc // P
    # projection e-chunks (PSUM free dim is 512 f32 max)
    chunks = []
    off = 0
    while off < ne_loc:
        w = min(512, ne_loc - off)
        chunks.append((off, w))
        off += w

    nc = bass.Bass("TRN2", target_bir_lowering=False, debug=False,
                   num_devices=N_CORES)

    ev_d = nc.dram_tensor("ev", [P, ET, NHID], bf16, kind="ExternalInput").ap()
    evT_d = nc.dram_tensor("evT", [P, KT, ne_loc], bf16, kind="ExternalInput").ap()
    clT_d = nc.dram_tensor("clT", [P, KT, NC_LOC], bf16, kind="ExternalInput").ap()
    weh_d = nc.dram_tensor("weh", [PROJ, ne_loc], bf16, kind="ExternalInput").ap()
    wch_d = nc.dram_tensor("wch", [PROJ, NC_LOC], bf16, kind="ExternalInput").ap()
    web_d = nc.dram_tensor("web", [P, KT, PROJ], bf16, kind="ExternalInput").ap()
    wcb_d = nc.dram_tensor("wcb", [P, KT, PROJ], bf16, kind="ExternalInput").ap()
    wab_d = nc.dram_tensor("wab", [P, KO, NHID], bf16, kind="ExternalInput").ap()
    bc_d = nc.dram_tensor("bc", [PROJ, 1], f32, kind="ExternalInput").ap()
    be_d = nc.dram_tensor("be", [PROJ, 1], f32, kind="ExternalInput").ap()
    ohs_d = nc.dram_tensor("ohs", [P, CT, NG], bf16, kind="ExternalInput").ap()
    seg_d = nc.dram_tensor("seg", [NG, NHID], f32, kind="ExternalOutput").ap()

    with _PatchedTileContext(nc) as tc:
        with (
            tc.tile_pool(name="const", bufs=1) as cpool,
            tc.tile_pool(name="big", bufs=1) as bigpool,
            # PSUM banks: s-ring 2 + pv 4 + rowsum 1 = 7 of 8
            tc.tile_pool(name="psS", bufs=2, space="PSUM") as psS,
            tc.tile_pool(name="psV", bufs=1, space="PSUM") as psV,
        ):
            # ---------- persistent constants (loaded once) ----------
            ev_sb = cpool.tile([P, ET, NHID], bf16)
            nc.sync.dma_start(ev_sb[:], ev_d[:])
            evT_sb = cpool.tile([P, KT, ne_loc], bf16)
            nc.sync.dma_start(evT_sb[:], evT_d[:])
            clT_sb = cpool.tile([P, KT, NC_LOC], bf16)
            nc.sync.dma_start(clT_sb[:], clT_d[:])
            web_sb = cpool.tile([P, KT, PROJ], bf16)
            nc.sync.dma_start(web_sb[:], web_d[:])
            wcb_sb = cpool.tile([P, KT, PROJ], bf16)
            nc.sync.dma_start(wcb_sb[:], wcb_d[:])
            wab_sb = cpool.tile([P, KO, NHID], bf16)
            nc.sync.dma_start(wab_sb[:], wab_d[:])
            bc_sb = cpool.tile([PROJ, 1], f32)
            nc.sync.dma_start(bc_sb[:], bc_d[:])
            be_sb = cpool.tile([PROJ, 1], f32)
            nc.sync.dma_start(be_sb[:], be_d[:])
            ohs_sb = cpool.tile([P, CT, NG], bf16)
            nc.sync.dma_start(ohs_sb[:], ohs_d[:])
            exp_bias = cpool.tile([P, 1], f32)
            nc.gpsimd.memset(exp_bias[:], EXP_BIAS)
            ones_row = cpool.tile([1, P], bf16)
            nc.gpsimd.memset(ones_row[:], 1.0)
            ones_col = cpool.tile([P, 1], bf16)
            nc.gpsimd.memset(ones_col[:], 1.0)

            with tc.tile_pool(name="work", bufs=2) as wpool:
                for rep in range(reps):
                    # ---------- projections (+ one-hot halves from host) ----------
                    we_aug = wpool.tile([P, ne_loc], bf16, tag="weaug")
                    nc.sync.dma_start(we_aug[PROJ:, :], weh_d[:])
                    for off, w in chunks:
                        ps = psS.tile([P, NHID], f32, tag="s")
                        for k in range(KT):
                            nc.tensor.matmul(ps[:PROJ, :w], web_sb[:, k, :],
                                             evT_sb[:, k, off:off + w],
                                             start=(k == 0), stop=(k == KT - 1))
                        nc.scalar.activation(we_aug[:PROJ, off:off + w],
                                             ps[:PROJ, :w], AF.Identity,
                                             bias=be_sb[:])
                    wc_aug = wpool.tile([P, NC_LOC], bf16, tag="wcaug")
                    nc.sync.dma_start(wc_aug[PROJ:, :], wch_d[:])
                    ps = psS.tile([P, NHID], f32, tag="s")
                    for k in range(KT):
                        nc.tensor.matmul(ps[:PROJ, :], wcb_sb[:, k, :],
                                         clT_sb[:, k, :],
                                         start=(k == 0), stop=(k == KT - 1))
                    nc.scalar.activation(wc_aug[:PROJ, :], ps[:PROJ, :],
                                         AF.Identity, bias=bc_sb[:])

                    # ---------- scores^T -> exp -> PV pipeline ----------
                    p_sb = wpool.tile([P, ET, NHID], bf16, tag="p")
                    pvs = [psV.tile([P, NC_LOC], f32, tag=f"pv{h}",
                                    name=f"pv{h}_{rep}") for h in range(KT)]
                    rs_ps = psV.tile([1, NC_LOC], f32, tag="rs",
                                     name=f"rs_{rep}")
                    for e in range(ET):
                        s_ps = psS.tile([P, NHID], f32, tag="s")
                        nc.tensor.matmul(s_ps[:], we_aug[:, e * P:(e + 1) * P],
                                         wc_aug[:], start=True, stop=True)
                        nc.scalar.activation(p_sb[:, e, :], s_ps[:], AF.Exp,
                                             bias=exp_bias[:])
                        for h in range(KT):
                            nc.tensor.matmul(pvs[h][:],
                                             ev_sb[:, e, h * P:(h + 1) * P],
                                             p_sb[:, e, :],
                                             start=(e == 0), stop=(e == ET - 1))
                        nc.tensor.matmul(rs_ps[:], ones_col[:], p_sb[:, e, :],
                                         start=(e == 0), stop=(e == ET - 1))

                    # ---------- normalize: recip + broadcast ----------
                    rs_sb = wpool.tile([1, NC_LOC], f32, tag="rssb")
                    nc.scalar.copy(rs_sb[:], rs_ps[:])
                    recip = wpool.tile([1, NC_LOC], f32, tag="recip")
                    nc.vector.reciprocal(recip[:], rs_sb[:])
                    recip_b = wpool.tile([1, NC_LOC], bf16, tag="recipb")
                    nc.vector.tensor_copy(recip_b[:], recip[:])
                    rbc_ps = psS.tile([P, NHID], f32, tag="s")
                    nc.tensor.matmul(rbc_ps[:, :NC_LOC], ones_row[:], recip_b[:],
                                     start=True, stop=True)
                    rbc = wpool.tile([P, NC_LOC], bf16, tag="rbc")
                    nc.scalar.copy(rbc[:], rbc_ps[:, :NC_LOC])

                    # ---------- aT blocks: cn^T and clT*cn^T ----------
                    cnT = wpool.tile([P, KT, NC_LOC], bf16, tag="cnT")
                    mlT = wpool.tile([P, KT, NC_LOC], bf16, tag="mlT")
                    for h in range(KT):
                        nc.vector.tensor_tensor(out=cnT[:, h, :], in0=pvs[h][:],
                                                in1=rbc[:], op=ALU.mult)
                        nc.vector.tensor_tensor(out=mlT[:, h, :],
                                                in0=clT_sb[:, h, :],
                                                in1=cnT[:, h, :], op=ALU.mult)

                    # ---------- a = aT^T @ Wa' ----------
                    a_out = wpool.tile([P, CT, NHID], bf16, tag="aout")
                    blocks = ([clT_sb[:, h, :] for h in range(KT)]
                              + [cnT[:, h, :] for h in range(KT)]
                              + [mlT[:, h, :] for h in range(KT)])
                    for t in range(CT):
                        o_ps = psS.tile([P, NHID], f32, tag="s")
                        for j, blk in enumerate(blocks):
                            nc.tensor.matmul(o_ps[:], blk[:, t * P:(t + 1) * P],
                                             wab_sb[:, j, :], start=(j == 0),
                                             stop=(j == KO - 1))
                        nc.scalar.copy(a_out[:, t, :], o_ps[:])

                    # ---------- segment sum via one-hot matmul ----------
                    seg_ps = psS.tile([P, NHID], f32, tag="s")
                    for t in range(CT):
                        nc.tensor.matmul(seg_ps[:NG, :], ohs_sb[:, t, :],
                                         a_out[:, t, :], start=(t == 0),
                                         stop=(t == CT - 1))
                    seg_sb = wpool.tile([NG, NHID], f32, tag="segsb")
                    nc.scalar.copy(seg_sb[:], seg_ps[:NG, :])
                    nc.sync.dma_start(seg_d[:], seg_sb[:])
    _split_excess_waits(nc)
    return nc


def make_in_maps(inputs: dict) -> tuple[list[dict], np.ndarray, np.ndarray, int]:
    """Host-side sharding: sort claims+evidence by graph, pre-gather x rows
    (bf16) into per-core contiguous slices tiled in SBUF layouts."""
    batch = np.asarray(inputs["batch"]).astype(np.int64)
    ci = np.asarray(inputs["claim_index"]).astype(np.int64)
    ei = np.asarray(inputs["evidence_index"]).astype(np.int64)
    x = np.asarray(inputs["x"], dtype=np.float32)
    cb = batch[ci]
    eb = batch[ei]
    counts = np.bincount(cb, minlength=NG).astype(np.float32)
    ba = np.asarray(inputs["ba"], dtype=np.float32).reshape(NHID)

    order_c = np.argsort(cb, kind="stable")
    cb_s = cb[order_c]
    order_e = np.argsort(eb, kind="stable")
    eb_s = eb[order_e]

    x_bf = x.astype(nbf16)
    xc = x_bf[ci[order_c]]          # [4096, 512] sorted claims
    xe = x_bf[ei[order_e]]          # [8192, 512] sorted evidence
    ev_starts = np.searchsorted(eb_s, np.arange(NG + 1))

    # per-core contiguous evidence spans
    spans = []
    for c in range(N_CORES):
        g_lo = int(cb_s[c * NC_LOC])
        g_hi = int(cb_s[(c + 1) * NC_LOC - 1])
        lo, hi = int(ev_starts[g_lo]), int(ev_starts[g_hi + 1])
        spans.append((lo, hi))
    ne_loc = max(512, -(-max(hi - lo for lo, hi in spans) // P) * P)
    ne_loc = min(ne_loc, NE)
    ET = ne_loc // P

    Wc = np.asarray(inputs["Wc"], dtype=np.float32)
    We = np.asarray(inputs["We"], dtype=np.float32)
    Wa = np.asarray(inputs["Wa"], dtype=np.float32)
    W1 = Wa[0:NHID] + Wa[2 * NHID:3 * NHID]
    W2 = Wa[NHID:2 * NHID] - Wa[2 * NHID:3 * NHID]
    W3 = Wa[3 * NHID:4 * NHID]
    wab = np.concatenate([W1, W2, W3], axis=0).astype(nbf16)  # [1536, 512]

    def tile_kpm(w, k):  # [(k p), m] -> [p, k, m]
        return np.ascontiguousarray(
            w.reshape(k, P, -1).transpose(1, 0, 2))

    g_ids = np.arange(NG)
    common = {
        "web": tile_kpm(We.astype(nbf16), KT),
        "wcb": tile_kpm(Wc.astype(nbf16), KT),
        "wab": tile_kpm(wab, KO),
        "bc": np.asarray(inputs["bc"], dtype=np.float32).reshape(PROJ, 1),
        "be": np.asarray(inputs["be"], dtype=np.float32).reshape(PROJ, 1),
    }
    in_maps = []
    for c in range(N_CORES):
        lo, hi = spans[c]
        lo = min(lo, NE - ne_loc)
        xe_c = xe[lo:lo + ne_loc]                  # [ne_loc, 512]
        eb_c = eb_s[lo:lo + ne_loc]
        xc_c = xc[c * NC_LOC:(c + 1) * NC_LOC]     # [512, 512]
        cb_c = cb_s[c * NC_LOC:(c + 1) * NC_LOC]
        m = dict(common)
        m["ev"] = np.ascontiguousarray(
            xe_c.reshape(ET, P, NHID).transpose(1, 0, 2))
        m["evT"] = np.ascontiguousarray(
            xe_c.T.reshape(KT, P, ne_loc).transpose(1, 0, 2))
        m["clT"] = np.ascontiguousarray(
            xc_c.T.reshape(KT, P, NC_LOC).transpose(1, 0, 2))
        m["weh"] = (MAG * (eb_c[None, :] == g_ids[:PROJ, None])).astype(nbf16)
        m["wch"] = (MAG * (cb_c[None, :] == g_ids[:PROJ, None])).astype(nbf16)
        m["ohs"] = np.ascontiguousarray(
            (cb_c.reshape(CT, P)[:, :, None] == g_ids[None, None, :])
            .transpose(1, 0, 2)).astype(nbf16)
        in_maps.append(m)
    return in_maps, counts, ba, ne_loc


def postprocess(results: list, counts: np.ndarray, ba: np.ndarray) -> np.ndarray:
    seg = np.zeros((NG, NHID), np.float64)
    for c in range(N_CORES):
        seg += results[c]["seg"].astype(np.float64)
    # segment_mean(a + ba) = segment_mean(a) + ba, except empty graphs stay 0
    out = seg / np.maximum(counts, 1.0)[:, None] + (counts > 0)[:, None] * ba[None, :]
    return out.astype(np.float32)


def kernel(**inputs) -> np.ndarray:
    in_maps, counts, ba, ne_loc = make_in_maps(inputs)
    nc = build_nc(ne_loc=ne_loc)
    res = run_bass_kernel_spmd(nc, in_maps, list(range(N_CORES)))
    return postprocess(res.results, counts, ba)


# revision 5
# speedup vs baseline: 1.0787x; 1.0787x over previous
"""Trainium2 Bass kernel for the CrossAttention reference module.

  claim = x[claim_index]; evidence = x[evidence_index]
  wc = claim @ Wc + bc; we = evidence @ We + be
  S = wc @ we.T + blockdiag_mask(batch[claim_index], batch[evidence_index])
  A = softmax(S, -1); cn = A @ evidence
  a = concat([claim, cn, claim-cn, claim*cn]) @ Wa + ba
  out = segment_mean(a, batch[claim_index], 64)

Sharding (per the hint: devices hold gathered claim/evidence rows):
claims are sorted by graph and split 512 per core across 8 cores.
Evidence is sorted by graph on the host, so each core's evidence set is a
CONTIGUOUS slice of the sorted-evidence matrix (max span 1236 rows for
these inputs -> NE_LOC rounded to 1280).  The host pre-gathers x rows into
bf16 matrices tiled in the exact SBUF layouts the kernel wants (row-major
and transposed), so the device does plain contiguous DMAs: no gathers, no
on-device transposes, no dtype casts.

Device-side math (per core), all matmuls bf16 with fp32 PSUM accumulation:
  we_p^T = We^T @ evT + be            [64, NE_LOC]   (+ 64 one-hot rows from host)
  wc_p^T = Wc^T @ clT + bc            [64, 512]      (+ 64 one-hot rows)
  S^T[e-tile] = we_aug[:,e128]^T . wc_aug   -> [128e, 512c] per tile
  P^T = exp(S^T - 1074)               (one-hot trick adds +1024 on same-graph
                                       pairs; exp underflows to exactly 0 for
                                       cross-graph pairs, and the -50 shift is
                                       row-constant so softmax ignores it)
  cn^T[h] += ev[e,h128]^T . P^T       (PV with NO transposes: evidence rows are
  rowsum  += ones^T . P^T              the contraction dim in S^T space)
  cn^T *= 1/rowsum (broadcast via ones-matmul)
  a = clT^T.W1' + cnT^T.W2' + (clT*cnT)^T.W3'   (Wa folded on host:
        W1'=Wa0+Wa2, W2'=Wa1-Wa2, W3'=Wa3 -- the claim-cn concat block folds away)
  seg = onehot(graph)^T . a           (segment sums; host divides by counts
                                       and adds ba: mean(a+ba)=mean(a)+ba)
"""

import sys

if "/opt/trn_rl_repo" not in sys.path:
    sys.path.insert(0, "/opt/trn_rl_repo")

import ml_dtypes
import numpy as np

import concourse.bass as bass
import concourse.mybir as mybir
import concourse.tile as tile
from concourse.bass_utils import run_bass_kernel_spmd
from concourse.vector_clock import ScopedClock

P = 128
NHID = 512
PROJ = 64
NC_ALL = 4096
NE = 8192
NG = 64
N_CORES = 8
NC_LOC = NC_ALL // N_CORES  # 512 claims per core
KT = NHID // P              # 4 hidden k-tiles
CT = NC_LOC // P            # 4 claim tiles per core
KO = 12                     # folded concat k-tiles (3 blocks x 4)
MAG = 32.0                  # sqrt(1024): one-hot scale
EXP_BIAS = -(MAG * MAG + 50.0)  # exp(S + 1024 - 1074) = exp(S - 50)

f32 = mybir.dt.float32
bf16 = mybir.dt.bfloat16
AF = mybir.ActivationFunctionType
ALU = mybir.AluOpType
nbf16 = ml_dtypes.bfloat16


class _PatchedTileContext(tile.TileContext):
    """Workaround: this neuronxcc/walrus build rejects InstDrain carrying
    sync waits ("Too many sync wait commands").  Collect the final drain's
    waits on nops (one wait each) and emit the drain itself wait-free.
    Also slimmed teardown: one barrier instead of two (the second barrier
    only guarded re-execution racing the sem clear, which NRT's serialized
    executions already prevent)."""

    def _drain_and_barrier(self, tick_clock, wait_clock):
        nc = self.nc
        nop0 = nc.sync.nop(nofuse=True)
        wait_clock.add_sem_waits(nop0.ins, ScopedClock({None: tick_clock.global_clock}))
        si = nop0.ins.sync_info
        waits = list(si.on_wait) if si and si.on_wait else []
        if si and len(waits) > 1:
            del si.on_wait[1:]
            for w in waits[1:]:
                extra = nc.sync.nop(nofuse=True)
                if extra.ins.sync_info is None:
                    extra.ins.sync_info = mybir.SyncInfo(on_wait=[w], on_update=[])
                else:
                    extra.ins.sync_info.on_wait.append(w)
        drain_inst = nc.sync.drain()
        wait_clock.add_sem_waits(
            drain_inst.ins, ScopedClock({None: tick_clock.global_clock})
        )
        dsi = drain_inst.ins.sync_info
        if dsi and dsi.on_wait:
            del dsi.on_wait[:]
        nc.all_engine_barrier()
        popped = nc._tile_sem_poison_stack.pop()
        assert popped is self._sem_poison
        nc.clear_and_free_semaphores(list(self.sems.allocated().values()))


def _split_excess_waits(nc: bass.Bass, limit: int = 1) -> None:
    """This walrus build rejects instructions carrying more than ~1 sync
    wait.  Move excess waits onto injected same-engine nops (engines are
    in-order, so gating a preceding nop gates the instruction)."""
    for f in nc.m.functions:
        for bb in f.blocks:
            new_insts = []
            for inst in bb.instructions:
                si = getattr(inst, "sync_info", None)
                if si is not None and si.on_wait and len(si.on_wait) > limit:
                    keep = list(si.on_wait[-limit:])
                    excess = list(si.on_wait[:-limit])
                    for w in excess:
                        nop = mybir.InstNoOp(
                            name=f"I-{nc.next_id()}", engine=inst.engine,
                            ins=[], outs=[],
                            sync_info=mybir.SyncInfo(on_wait=[w], on_update=[]))
                        new_insts.append(nop)
                    del si.on_wait[:]
                    si.on_wait.extend(keep)
                new_insts.append(inst)
            bb.instructions[:] = new_insts


def build_nc(ne_loc: int = 1280, reps: int = 1) -> bass.Bass:
    assert ne_loc % P == 0
    ET = ne_loc // P
    # projection e-chunks (PSUM free dim is 512 f32 max)
    chunks = []
    off = 0
    while off < ne_loc:
        w = min(512, ne_loc - off)
        chunks.append((off, w))
        off += w

    nc = bass.Bass("TRN2", target_bir_lowering=False, debug=False,
                   num_devices=N_CORES)

    ev_d = nc.dram_tensor("ev", [P, ET, NHID], bf16, kind="ExternalInput").ap()
    evT_d = nc.dram_tensor("evT", [P, KT, ne_loc], bf16, kind="ExternalInput").ap()
    clT_d = nc.dram_tensor("clT", [P, KT, NC_LOC], bf16, kind="ExternalInput").ap()
    weh_d = nc.dram_tensor("weh", [PROJ, ne_loc], bf16, kind="ExternalInput").ap()
    wch_d = nc.dram_tensor("wch", [PROJ, NC_LOC], bf16, kind="ExternalInput").ap()
    web_d = nc.dram_tensor("web", [P, KT, PROJ], bf16, kind="ExternalInput").ap()
    wcb_d = nc.dram_tensor("wcb", [P, KT, PROJ], bf16, kind="ExternalInput").ap()
    wab_d = nc.dram_tensor("wab", [P, KO, NHID], bf16, kind="ExternalInput").ap()
    bc_d = nc.dram_tensor("bc", [PROJ, 1], f32, kind="ExternalInput").ap()
    be_d = nc.dram_tensor("be", [PROJ, 1], f32, kind="ExternalInput").ap()
    ohs_d = nc.dram_tensor("ohs", [P, CT, NG], bf16, kind="ExternalInput").ap()
    seg_d = nc.dram_tensor("seg", [NG, NHID], f32, kind="ExternalOutput").ap()

    N_WARM = 8  # dummy matmuls during the DMA prologue flip HAM to 2.4 GHz

    with _PatchedTileContext(nc) as tc:
        with (
            tc.tile_pool(name="const", bufs=1) as cpool,
            # PSUM banks: s-ring 2 + pv 4 + rowsum 1 + warm 1 = 8 of 8
            tc.tile_pool(name="psS", bufs=2, space="PSUM") as psS,
            tc.tile_pool(name="psV", bufs=1, space="PSUM") as psV,
            tc.tile_pool(name="psW", bufs=1, space="PSUM") as psW,
        ):
            # ---------- constants, in first-use order ----------
            # gpsimd memsets first (independent of DMA)
            exp_bias = cpool.tile([P, 1], f32)
            nc.gpsimd.memset(exp_bias[:], EXP_BIAS)
            ones_mat = cpool.tile([P, P], bf16)
            nc.gpsimd.memset(ones_mat[:], 1.0)
            scratch = cpool.tile([P, NHID], bf16)
            nc.gpsimd.memset(scratch[:], 0.5)
            # sync-issued DMAs (HWDGE), smallest/earliest-needed first
            web_sb = cpool.tile([P, KT, PROJ], bf16)
            nc.sync.dma_start(web_sb[:], web_d[:])
            wcb_sb = cpool.tile([P, KT, PROJ], bf16)
            nc.sync.dma_start(wcb_sb[:], wcb_d[:])
            bc_sb = cpool.tile([PROJ, 1], f32)
            nc.sync.dma_start(bc_sb[:], bc_d[:])
            be_sb = cpool.tile([PROJ, 1], f32)
            nc.sync.dma_start(be_sb[:], be_d[:])
            clT_sb = cpool.tile([P, KT, NC_LOC], bf16)
            nc.sync.dma_start(clT_sb[:], clT_d[:])
            evT_sb = cpool.tile([P, KT, ne_loc], bf16)
            for off, w in chunks:
                nc.sync.dma_start(evT_sb[:, :, off:off + w],
                                  evT_d[:, :, off:off + w])
            # scalar-issued DMAs (also HWDGE) — needed later, issued in
            # parallel so the sync queue stays short
            ev_sb = cpool.tile([P, ET, NHID], bf16)
            half = ET // 2
            nc.scalar.dma_start(ev_sb[:, :half, :], ev_d[:, :half, :])
            nc.scalar.dma_start(ev_sb[:, half:, :], ev_d[:, half:, :])
            wab_sb = cpool.tile([P, KO, NHID], bf16)
            nc.scalar.dma_start(wab_sb[:], wab_d[:])
            ohs_sb = cpool.tile([P, CT, NG], bf16)
            nc.scalar.dma_start(ohs_sb[:], ohs_d[:])

            # ---------- HAM warm-up: garbage matmuls during DMA wait ----------
            warm_ps = psW.tile([P, NHID], f32, tag="warm")
            for i in range(N_WARM):
                nc.tensor.matmul(warm_ps[:], scratch[:, :P], scratch[:],
                                 start=True, stop=True)

            with tc.tile_pool(name="work", bufs=2) as wpool:
                for rep in range(reps):
                    # ---------- projections (+ one-hot halves from host) ----------
                    wc_aug = wpool.tile([P, NC_LOC], bf16, tag="wcaug")
                    nc.sync.dma_start(wc_aug[PROJ:, :], wch_d[:])
                    ps = psS.tile([P, NHID], f32, tag="s")
                    for k in range(KT):
                        nc.tensor.matmul(ps[:PROJ, :], wcb_sb[:, k, :],
                                         clT_sb[:, k, :],
                                         start=(k == 0), stop=(k == KT - 1))
                    nc.scalar.activation(wc_aug[:PROJ, :], ps[:PROJ, :],
                                         AF.Identity, bias=bc_sb[:])
                    we_aug = wpool.tile([P, ne_loc], bf16, tag="weaug")
                    nc.sync.dma_start(we_aug[PROJ:, :], weh_d[:])
                    for off, w in chunks:
                        ps = psS.tile([P, NHID], f32, tag="s")
                        for k in range(KT):
                            nc.tensor.matmul(ps[:PROJ, :w], web_sb[:, k, :],
                                             evT_sb[:, k, off:off + w],
                                             start=(k == 0), stop=(k == KT - 1))
                        nc.scalar.activation(we_aug[:PROJ, off:off + w],
                                             ps[:PROJ, :w], AF.Identity,
                                             bias=be_sb[:])

                    # ---------- scores^T -> exp -> PV pipeline ----------
                    p_sb = wpool.tile([P, ET, NHID], bf16, tag="p")
                    pvs = [psV.tile([P, NC_LOC], f32, tag=f"pv{h}",
                                    name=f"pv{h}_{rep}") for h in range(KT)]
                    rs_ps = psV.tile([P, NC_LOC], f32, tag="rs",
                                     name=f"rs_{rep}")
                    for e in range(ET):
                        s_ps = psS.tile([P, NHID], f32, tag="s")
                        nc.tensor.matmul(s_ps[:], we_aug[:, e * P:(e + 1) * P],
                                         wc_aug[:], start=True, stop=True)
                        nc.scalar.activation(p_sb[:, e, :], s_ps[:], AF.Exp,
                                             bias=exp_bias[:])
                        for h in range(KT):
                            nc.tensor.matmul(pvs[h][:],
                                             ev_sb[:, e, h * P:(h + 1) * P],
                                             p_sb[:, e, :],
                                             start=(e == 0), stop=(e == ET - 1))
                        # rowsum, pre-broadcast to all 128 partitions
                        nc.tensor.matmul(rs_ps[:], ones_mat[:], p_sb[:, e, :],
                                         start=(e == 0), stop=(e == ET - 1))

                    # ---------- normalize: 1/rs = exp(-ln(rs)) on ScalarE ----------
                    # (DVE reciprocal is an 8-cycle iterative op: 3.3us for 512
                    # elements; ACT ln+exp streams at 1 elem/cycle/lane and both
                    # live in the natural_log_exp_and_others table set)
                    lnr = wpool.tile([P, NC_LOC], f32, tag="lnr")
                    nc.scalar.activation(lnr[:], rs_ps[:], AF.Ln)
                    rbc = wpool.tile([P, NC_LOC], bf16, tag="rbc")
                    nc.scalar.activation(rbc[:], lnr[:], AF.Exp, scale=-1.0)

                    # ---------- aT blocks: cn^T and clT*cn^T ----------
                    cnT = wpool.tile([P, KT, NC_LOC], bf16, tag="cnT")
                    mlT = wpool.tile([P, KT, NC_LOC], bf16, tag="mlT")
                    for h in range(KT):
                        nc.vector.tensor_tensor(out=cnT[:, h, :], in0=pvs[h][:],
                                                in1=rbc[:], op=ALU.mult)
                        nc.vector.tensor_tensor(out=mlT[:, h, :],
                                                in0=clT_sb[:, h, :],
                                                in1=cnT[:, h, :], op=ALU.mult)

                    # ---------- a = aT^T @ Wa' ----------
                    a_out = wpool.tile([P, CT, NHID], bf16, tag="aout")
                    blocks = ([clT_sb[:, h, :] for h in range(KT)]
                              + [cnT[:, h, :] for h in range(KT)]
                              + [mlT[:, h, :] for h in range(KT)])
                    for t in range(CT):
                        o_ps = psS.tile([P, NHID], f32, tag="s")
                        for j, blk in enumerate(blocks):
                            nc.tensor.matmul(o_ps[:], blk[:, t * P:(t + 1) * P],
                                             wab_sb[:, j, :], start=(j == 0),
                                             stop=(j == KO - 1))
                        nc.scalar.copy(a_out[:, t, :], o_ps[:])

                    # ---------- segment sum via one-hot matmul ----------
                    seg_ps = psS.tile([P, NHID], f32, tag="s")
                    for t in range(CT):
                        nc.tensor.matmul(seg_ps[:NG, :], ohs_sb[:, t, :],
                                         a_out[:, t, :], start=(t == 0),
                                         stop=(t == CT - 1))
                    seg_sb = wpool.tile([NG, NHID], f32, tag="segsb")
                    nc.scalar.copy(seg_sb[:], seg_ps[:NG, :])
                    nc.sync.dma_start(seg_d[:], seg_sb[:])
    _split_excess_waits(nc)
    return nc


def make_in_maps(inputs: dict) -> tuple[list[dict], np.ndarray, np.ndarray, int]:
    """Host-side sharding: sort claims+evidence by graph, pre-gather x rows
    (bf16) into per-core contiguous slices tiled in SBUF layouts."""
    batch = np.asarray(inputs["batch"]).astype(np.int64)
    ci = np.asarray(inputs["claim_index"]).astype(np.int64)
    ei = np.asarray(inputs["evidence_index"]).astype(np.int64)
    x = np.asarray(inputs["x"], dtype=np.float32)
    cb = batch[ci]
    eb = batch[ei]
    counts = np.bincount(cb, minlength=NG).astype(np.float32)
    ba = np.asarray(inputs["ba"], dtype=np.float32).reshape(NHID)

    order_c = np.argsort(cb, kind="stable")
    cb_s = cb[order_c]
    order_e = np.argsort(eb, kind="stable")
    eb_s = eb[order_e]

    x_bf = x.astype(nbf16)
    xc = x_bf[ci[order_c]]          # [4096, 512] sorted claims
    xe = x_bf[ei[order_e]]          # [8192, 512] sorted evidence
    ev_starts = np.searchsorted(eb_s, np.arange(NG + 1))

    # per-core contiguous evidence spans
    spans = []
    for c in range(N_CORES):
        g_lo = int(cb_s[c * NC_LOC])
        g_hi = int(cb_s[(c + 1) * NC_LOC - 1])
        lo, hi = int(ev_starts[g_lo]), int(ev_starts[g_hi + 1])
        spans.append((lo, hi))
    ne_loc = max(512, -(-max(hi - lo for lo, hi in spans) // P) * P)
    ne_loc = min(ne_loc, NE)
    ET = ne_loc // P

    Wc = np.asarray(inputs["Wc"], dtype=np.float32)
    We = np.asarray(inputs["We"], dtype=np.float32)
    Wa = np.asarray(inputs["Wa"], dtype=np.float32)
    W1 = Wa[0:NHID] + Wa[2 * NHID:3 * NHID]
    W2 = Wa[NHID:2 * NHID] - Wa[2 * NHID:3 * NHID]
    W3 = Wa[3 * NHID:4 * NHID]
    wab = np.concatenate([W1, W2, W3], axis=0).astype(nbf16)  # [1536, 512]

    def tile_kpm(w, k):  # [(k p), m] -> [p, k, m]
        return np.ascontiguousarray(
            w.reshape(k, P, -1).transpose(1, 0, 2))

    g_ids = np.arange(NG)
    common = {
        "web": tile_kpm(We.astype(nbf16), KT),
        "wcb": tile_kpm(Wc.astype(nbf16), KT),
        "wab": tile_kpm(wab, KO),
        "bc": np.asarray(inputs["bc"], dtype=np.float32).reshape(PROJ, 1),
        "be": np.asarray(inputs["be"], dtype=np.float32).reshape(PROJ, 1),
    }
    in_maps = []
    for c in range(N_CORES):
        lo, hi = spans[c]
        lo = min(lo, NE - ne_loc)
        xe_c = xe[lo:lo + ne_loc]                  # [ne_loc, 512]
        eb_c = eb_s[lo:lo + ne_loc]
        xc_c = xc[c * NC_LOC:(c + 1) * NC_LOC]     # [512, 512]
        cb_c = cb_s[c * NC_LOC:(c + 1) * NC_LOC]
        m = dict(common)
        m["ev"] = np.ascontiguousarray(
            xe_c.reshape(ET, P, NHID).transpose(1, 0, 2))
        m["evT"] = np.ascontiguousarray(
            xe_c.T.reshape(KT, P, ne_loc).transpose(1, 0, 2))
        m["clT"] = np.ascontiguousarray(
            xc_c.T.reshape(KT, P, NC_LOC).transpose(1, 0, 2))
        m["weh"] = (MAG * (eb_c[None, :] == g_ids[:PROJ, None])).astype(nbf16)
        m["wch"] = (MAG * (cb_c[None, :] == g_ids[:PROJ, None])).astype(nbf16)
        m["ohs"] = np.ascontiguousarray(
            (cb_c.reshape(CT, P)[:, :, None] == g_ids[None, None, :])
            .transpose(1, 0, 2)).astype(nbf16)
        in_maps.append(m)
    return in_maps, counts, ba, ne_loc


def postprocess(results: list, counts: np.ndarray, ba: np.ndarray) -> np.ndarray:
    seg = np.zeros((NG, NHID), np.float64)
    for c in range(N_CORES):
        seg += results[c]["seg"].astype(np.float64)
    # segment_mean(a + ba) = segment_mean(a) + ba, except empty graphs stay 0
    out = seg / np.maximum(counts, 1.0)[:, None] + (counts > 0)[:, None] * ba[None, :]
    return out.astype(np.float32)


def kernel(**inputs) -> np.ndarray:
    in_maps, counts, ba, ne_loc = make_in_maps(inputs)
    nc = build_nc(ne_loc=ne_loc)
    res = run_bass_kernel_spmd(nc, in_maps, list(range(N_CORES)))
    return postprocess(res.results, counts, ba)


# revision 10
# speedup vs baseline: 1.2950x; 1.2005x over previous
"""Trainium2 Bass kernel for the CrossAttention reference module.

  claim = x[claim_index]; evidence = x[evidence_index]
  wc = claim @ Wc + bc; we = evidence @ We + be
  S = wc @ we.T + blockdiag_mask(batch[claim_index], batch[evidence_index])
  A = softmax(S, -1); cn = A @ evidence
  a = concat([claim, cn, claim-cn, claim*cn]) @ Wa + ba
  out = segment_mean(a, batch[claim_index], 64)

Sharding (per the hint: devices hold gathered claim/evidence rows):
claims are sorted by graph and split 512 per core across 8 cores.
Evidence is sorted by graph on the host, so each core's evidence set is a
CONTIGUOUS slice of the sorted-evidence matrix (max span 1236 rows for
these inputs -> NE_LOC rounded to 1280).  The host pre-gathers x rows into
bf16 matrices tiled in the exact SBUF layouts the kernel wants (row-major
and transposed), so the device does plain contiguous DMAs: no gathers, no
on-device transposes, no dtype casts.

Device-side math (per core), all matmuls bf16 with fp32 PSUM accumulation:
  we_p^T = We^T @ evT + be            [64, NE_LOC]   (+ 64 one-hot rows from host)
  wc_p^T = Wc^T @ clT + bc            [64, 512]      (+ 64 one-hot rows)
  S^T[e-tile] = we_aug[:,e128]^T . wc_aug   -> [128e, 512c] per tile
  P^T = exp(S^T - 1074)               (one-hot trick adds +1024 on same-graph
                                       pairs; exp underflows to exactly 0 for
                                       cross-graph pairs, and the -50 shift is
                                       row-constant so softmax ignores it)
  cn^T[h] += ev[e,h128]^T . P^T       (PV with NO transposes: evidence rows are
  rowsum  += ones^T . P^T              the contraction dim in S^T space)
  cn^T *= 1/rowsum (broadcast via ones-matmul)
  a = clT^T.W1' + cnT^T.W2' + (clT*cnT)^T.W3'   (Wa folded on host:
        W1'=Wa0+Wa2, W2'=Wa1-Wa2, W3'=Wa3 -- the claim-cn concat block folds away)
  seg = onehot(graph)^T . a           (segment sums; host divides by counts
                                       and adds ba: mean(a+ba)=mean(a)+ba)
"""

import sys

if "/opt/trn_rl_repo" not in sys.path:
    sys.path.insert(0, "/opt/trn_rl_repo")

import ml_dtypes
import numpy as np

import concourse.bass as bass
import concourse.mybir as mybir
import concourse.tile as tile
from concourse.bass_utils import run_bass_kernel_spmd
from concourse.vector_clock import ScopedClock

P = 128
NHID = 512
PROJ = 64
NC_ALL = 4096
NE = 8192
NG = 64
N_CORES = 8
NC_LOC = NC_ALL // N_CORES  # 512 claims per core
KT = NHID // P              # 4 hidden k-tiles
CT = NC_LOC // P            # 4 claim tiles per core
KO = 12                     # folded concat k-tiles (3 blocks x 4)
MAG = 32.0                  # sqrt(1024): one-hot scale
EXP_BIAS = -(MAG * MAG + 50.0)  # exp(S + 1024 - 1074) = exp(S - 50)

f32 = mybir.dt.float32
bf16 = mybir.dt.bfloat16
AF = mybir.ActivationFunctionType
ALU = mybir.AluOpType
nbf16 = ml_dtypes.bfloat16


class _PatchedTileContext(tile.TileContext):
    """Workaround: this neuronxcc/walrus build rejects InstDrain carrying
    sync waits ("Too many sync wait commands").  Collect the final drain's
    waits on nops (one wait each) and emit the drain itself wait-free.
    Also slimmed teardown: one barrier instead of two (the second barrier
    only guarded re-execution racing the sem clear, which NRT's serialized
    executions already prevent)."""

    def _drain_and_barrier(self, tick_clock, wait_clock):
        nc = self.nc
        nop0 = nc.sync.nop(nofuse=True)
        wait_clock.add_sem_waits(nop0.ins, ScopedClock({None: tick_clock.global_clock}))
        si = nop0.ins.sync_info
        waits = list(si.on_wait) if si and si.on_wait else []
        if si and len(waits) > 1:
            del si.on_wait[1:]
            for w in waits[1:]:
                extra = nc.sync.nop(nofuse=True)
                if extra.ins.sync_info is None:
                    extra.ins.sync_info = mybir.SyncInfo(on_wait=[w], on_update=[])
                else:
                    extra.ins.sync_info.on_wait.append(w)
        drain_inst = nc.sync.drain()
        wait_clock.add_sem_waits(
            drain_inst.ins, ScopedClock({None: tick_clock.global_clock})
        )
        dsi = drain_inst.ins.sync_info
        if dsi and dsi.on_wait:
            del dsi.on_wait[:]
        nc.all_engine_barrier()
        popped = nc._tile_sem_poison_stack.pop()
        assert popped is self._sem_poison
        nc.clear_and_free_semaphores(list(self.sems.allocated().values()))


def _split_excess_waits(nc: bass.Bass, limit: int = 1) -> None:
    """This walrus build rejects instructions carrying more than ~1 sync
    wait.  Move excess waits onto injected same-engine nops (engines are
    in-order, so gating a preceding nop gates the instruction)."""
    for f in nc.m.functions:
        for bb in f.blocks:
            new_insts = []
            for inst in bb.instructions:
                si = getattr(inst, "sync_info", None)
                if si is not None and si.on_wait and len(si.on_wait) > limit:
                    keep = list(si.on_wait[-limit:])
                    excess = list(si.on_wait[:-limit])
                    for w in excess:
                        nop = mybir.InstNoOp(
                            name=f"I-{nc.next_id()}", engine=inst.engine,
                            ins=[], outs=[],
                            sync_info=mybir.SyncInfo(on_wait=[w], on_update=[]))
                        new_insts.append(nop)
                    del si.on_wait[:]
                    si.on_wait.extend(keep)
                new_insts.append(inst)
            bb.instructions[:] = new_insts


def build_nc(ne_loc: int = 1280, reps: int = 1) -> bass.Bass:
    assert ne_loc % P == 0
    ET = ne_loc // P
    # projection e-chunks (PSUM free dim is 512 f32 max)
    chunks = []
    off = 0
    while off < ne_loc:
        w = min(512, ne_loc - off)
        chunks.append((off, w))
        off += w

    nc = bass.Bass("TRN2", target_bir_lowering=False, debug=False,
                   num_devices=N_CORES)

    ev_d = nc.dram_tensor("ev", [P, ET, NHID], bf16, kind="ExternalInput").ap()
    evT_d = nc.dram_tensor("evT", [P, KT, ne_loc], bf16, kind="ExternalInput").ap()
    clT_d = nc.dram_tensor("clT", [P, KT, NC_LOC], bf16, kind="ExternalInput").ap()
    weh_d = nc.dram_tensor("weh", [PROJ, ne_loc], bf16, kind="ExternalInput").ap()
    wch_d = nc.dram_tensor("wch", [PROJ, NC_LOC], bf16, kind="ExternalInput").ap()
    web_d = nc.dram_tensor("web", [P, KT, PROJ], bf16, kind="ExternalInput").ap()
    wcb_d = nc.dram_tensor("wcb", [P, KT, PROJ], bf16, kind="ExternalInput").ap()
    wab_d = nc.dram_tensor("wab", [P, KO, NHID], bf16, kind="ExternalInput").ap()
    bc_d = nc.dram_tensor("bc", [PROJ, 1], f32, kind="ExternalInput").ap()
    be_d = nc.dram_tensor("be", [PROJ, 1], f32, kind="ExternalInput").ap()
    ohs_d = nc.dram_tensor("ohs", [P, CT, NG], bf16, kind="ExternalInput").ap()
    seg_d = nc.dram_tensor("seg", [NG, NHID], f32, kind="ExternalOutput").ap()

    N_WARM = 4  # dummy matmuls during the DMA prologue flip HAM to 2.4 GHz

    with _PatchedTileContext(nc) as tc:
        with (
            tc.tile_pool(name="const", bufs=1) as cpool,
            # PSUM banks: s-ring 3 + pv 4 + rowsum 1 = 8 of 8
            tc.tile_pool(name="psS", bufs=3, space="PSUM") as psS,
            tc.tile_pool(name="psV", bufs=1, space="PSUM") as psV,
        ):
            # ---------- constants ----------
            # gpsimd memsets first (independent of DMA)
            scratch = cpool.tile([P, NHID], bf16)
            nc.gpsimd.memset(scratch[:], 0.5)
            exp_bias = cpool.tile([P, 1], f32)
            nc.gpsimd.memset(exp_bias[:], EXP_BIAS)
            ones_mat = cpool.tile([P, P], bf16)
            nc.gpsimd.memset(ones_mat[:], 1.0)
            # All loads on the sync HWDGE queue, in strict first-use order:
            # a single FIFO keeps DMA bandwidth on the critical-path tensor
            # instead of splitting it across concurrent queues.
            wcb_sb = cpool.tile([P, KT, PROJ], bf16)
            nc.sync.dma_start(wcb_sb[:], wcb_d[:])
            bc_sb = cpool.tile([PROJ, 1], f32)
            nc.sync.dma_start(bc_sb[:], bc_d[:])
            web_sb = cpool.tile([P, KT, PROJ], bf16)
            nc.sync.dma_start(web_sb[:], web_d[:])
            be_sb = cpool.tile([PROJ, 1], f32)
            nc.sync.dma_start(be_sb[:], be_d[:])
            clT_sb = cpool.tile([P, KT, NC_LOC], bf16)
            nc.sync.dma_start(clT_sb[:], clT_d[:])
            evT_sb = cpool.tile([P, KT, ne_loc], bf16)
            for off, w in chunks:
                nc.sync.dma_start(evT_sb[:, :, off:off + w],
                                  evT_d[:, :, off:off + w])
            wc_aug = cpool.tile([P, NC_LOC], bf16)
            nc.sync.dma_start(wc_aug[PROJ:, :], wch_d[:])
            we_aug = cpool.tile([P, ne_loc], bf16)
            nc.sync.dma_start(we_aug[PROJ:, :], weh_d[:])
            ev_sb = cpool.tile([P, ET, NHID], bf16)
            ev_splits = [(0, 2)] + [(a, min(a + 4, ET)) for a in range(2, ET, 4)]
            for a, b in ev_splits:
                nc.sync.dma_start(ev_sb[:, a:b, :], ev_d[:, a:b, :])
            wab_sb = cpool.tile([P, KO, NHID], bf16)
            nc.sync.dma_start(wab_sb[:], wab_d[:])
            ohs_sb = cpool.tile([P, CT, NG], bf16)
            nc.sync.dma_start(ohs_sb[:], ohs_d[:])

            # ---------- HAM warm-up: garbage matmuls during DMA wait ----------
            # (into the s-ring; no readers, so the ring never stalls on them)
            for i in range(N_WARM):
                warm_ps = psS.tile([P, NHID], f32, tag="s")
                nc.tensor.matmul(warm_ps[:], scratch[:, :P], scratch[:],
                                 start=True, stop=True)

            with tc.tile_pool(name="work", bufs=2) as wpool:
                for rep in range(reps):
                    # ---------- projections (+ one-hot halves from host) ----------
                    ps = psS.tile([P, NHID], f32, tag="s")
                    for k in range(KT):
                        nc.tensor.matmul(ps[:PROJ, :], wcb_sb[:, k, :],
                                         clT_sb[:, k, :],
                                         start=(k == 0), stop=(k == KT - 1))
                    nc.scalar.activation(wc_aug[:PROJ, :], ps[:PROJ, :],
                                         AF.Identity, bias=bc_sb[:])
                    for off, w in chunks:
                        ps = psS.tile([P, NHID], f32, tag="s")
                        for k in range(KT):
                            nc.tensor.matmul(ps[:PROJ, :w], web_sb[:, k, :],
                                             evT_sb[:, k, off:off + w],
                                             start=(k == 0), stop=(k == KT - 1))
                        nc.scalar.activation(we_aug[:PROJ, off:off + w],
                                             ps[:PROJ, :w], AF.Identity,
                                             bias=be_sb[:])

                    # ---------- scores^T -> exp -> PV pipeline ----------
                    p_sb = wpool.tile([P, ET, NHID], bf16, tag="p")
                    pvs = [psV.tile([P, NC_LOC], f32, tag=f"pv{h}",
                                    name=f"pv{h}_{rep}") for h in range(KT)]
                    rs_ps = psV.tile([P, NC_LOC], f32, tag="rs",
                                     name=f"rs_{rep}")
                    for e in range(ET):
                        s_ps = psS.tile([P, NHID], f32, tag="s")
                        nc.tensor.matmul(s_ps[:], we_aug[:, e * P:(e + 1) * P],
                                         wc_aug[:], start=True, stop=True)
                        nc.scalar.activation(p_sb[:, e, :], s_ps[:], AF.Exp,
                                             bias=exp_bias[:])
                        # rowsum first (pre-broadcast to 128 partitions): its
                        # stop fires before the pv tail, so the ln/exp
                        # normalize chain overlaps the last pv matmuls
                        nc.tensor.matmul(rs_ps[:], ones_mat[:], p_sb[:, e, :],
                                         start=(e == 0), stop=(e == ET - 1))
                        for h in range(KT):
                            nc.tensor.matmul(pvs[h][:],
                                             ev_sb[:, e, h * P:(h + 1) * P],
                                             p_sb[:, e, :],
                                             start=(e == 0), stop=(e == ET - 1))

                    # ---------- normalize: 1/rs = exp(-ln(rs)) on ScalarE ----------
                    # (DVE reciprocal is an 8-cycle iterative op: 3.3us for 512
                    # elements; ACT ln+exp streams at 1 elem/cycle/lane and both
                    # live in the natural_log_exp_and_others table set)
                    lnr = wpool.tile([P, NC_LOC], f32, tag="lnr")
                    nc.scalar.activation(lnr[:], rs_ps[:], AF.Ln)
                    rbc = wpool.tile([P, NC_LOC], bf16, tag="rbc")
                    nc.scalar.activation(rbc[:], lnr[:], AF.Exp, scale=-1.0)

                    # ---------- aT blocks: cn^T and clT*cn^T ----------
                    # DVE streams the 4 cn normalizations; the elementwise
                    # products go to GpSimd (idle, runs in parallel) except
                    # the last which DVE picks up after cn3.
                    cnT = wpool.tile([P, KT, NC_LOC], bf16, tag="cnT")
                    mlT = wpool.tile([P, KT, NC_LOC], bf16, tag="mlT")
                    for h in range(KT):
                        nc.vector.tensor_tensor(out=cnT[:, h, :], in0=pvs[h][:],
                                                in1=rbc[:], op=ALU.mult)
                    for h in range(KT):
                        eng = nc.vector if h == KT - 1 else nc.gpsimd
                        eng.tensor_tensor(out=mlT[:, h, :],
                                          in0=clT_sb[:, h, :],
                                          in1=cnT[:, h, :], op=ALU.mult)

                    # ---------- a = aT^T @ Wa' ----------
                    a_out = wpool.tile([P, CT, NHID], bf16, tag="aout")
                    blocks = ([clT_sb[:, h, :] for h in range(KT)]
                              + [cnT[:, h, :] for h in range(KT)]
                              + [mlT[:, h, :] for h in range(KT)])
                    for t in range(CT):
                        o_ps = psS.tile([P, NHID], f32, tag="s")
                        for j, blk in enumerate(blocks):
                            nc.tensor.matmul(o_ps[:], blk[:, t * P:(t + 1) * P],
                                             wab_sb[:, j, :], start=(j == 0),
                                             stop=(j == KO - 1))
                        nc.scalar.copy(a_out[:, t, :], o_ps[:])

                    # ---------- segment sum via one-hot matmul ----------
                    seg_ps = psS.tile([P, NHID], f32, tag="s")
                    for t in range(CT):
                        nc.tensor.matmul(seg_ps[:NG, :], ohs_sb[:, t, :],
                                         a_out[:, t, :], start=(t == 0),
                                         stop=(t == CT - 1))
                    seg_sb = wpool.tile([NG, NHID], f32, tag="segsb")
                    nc.scalar.copy(seg_sb[:], seg_ps[:NG, :])
                    nc.sync.dma_start(seg_d[:], seg_sb[:])
    _split_excess_waits(nc)
    return nc


def make_in_maps(inputs: dict) -> tuple[list[dict], np.ndarray, np.ndarray, int]:
    """Host-side sharding: sort claims+evidence by graph, pre-gather x rows
    (bf16) into per-core contiguous slices tiled in SBUF layouts."""
    batch = np.asarray(inputs["batch"]).astype(np.int64)
    ci = np.asarray(inputs["claim_index"]).astype(np.int64)
    ei = np.asarray(inputs["evidence_index"]).astype(np.int64)
    x = np.asarray(inputs["x"], dtype=np.float32)
    cb = batch[ci]
    eb = batch[ei]
    counts = np.bincount(cb, minlength=NG).astype(np.float32)
    ba = np.asarray(inputs["ba"], dtype=np.float32).reshape(NHID)

    order_c = np.argsort(cb, kind="stable")
    cb_s = cb[order_c]
    order_e = np.argsort(eb, kind="stable")
    eb_s = eb[order_e]

    x_bf = x.astype(nbf16)
    xc = x_bf[ci[order_c]]          # [4096, 512] sorted claims
    xe = x_bf[ei[order_e]]          # [8192, 512] sorted evidence
    ev_starts = np.searchsorted(eb_s, np.arange(NG + 1))

    # per-core contiguous evidence spans
    spans = []
    for c in range(N_CORES):
        g_lo = int(cb_s[c * NC_LOC])
        g_hi = int(cb_s[(c + 1) * NC_LOC - 1])
        lo, hi = int(ev_starts[g_lo]), int(ev_starts[g_hi + 1])
        spans.append((lo, hi))
    ne_loc = max(512, -(-max(hi - lo for lo, hi in spans) // P) * P)
    ne_loc = min(ne_loc, NE)
    ET = ne_loc // P

    Wc = np.asarray(inputs["Wc"], dtype=np.float32)
    We = np.asarray(inputs["We"], dtype=np.float32)
    Wa = np.asarray(inputs["Wa"], dtype=np.float32)
    W1 = Wa[0:NHID] + Wa[2 * NHID:3 * NHID]
    W2 = Wa[NHID:2 * NHID] - Wa[2 * NHID:3 * NHID]
    W3 = Wa[3 * NHID:4 * NHID]
    wab = np.concatenate([W1, W2, W3], axis=0).astype(nbf16)  # [1536, 512]

    def tile_kpm(w, k):  # [(k p), m] -> [p, k, m]
        return np.ascontiguousarray(
            w.reshape(k, P, -1).transpose(1, 0, 2))

    g_ids = np.arange(NG)
    common = {
        "web": tile_kpm(We.astype(nbf16), KT),
        "wcb": tile_kpm(Wc.astype(nbf16), KT),
        "wab": tile_kpm(wab, KO),
        "bc": np.asarray(inputs["bc"], dtype=np.float32).reshape(PROJ, 1),
        "be": np.asarray(inputs["be"], dtype=np.float32).reshape(PROJ, 1),
    }
    in_maps = []
    for c in range(N_CORES):
        lo, hi = spans[c]
        lo = min(lo, NE - ne_loc)
        xe_c = xe[lo:lo + ne_loc]                  # [ne_loc, 512]
        eb_c = eb_s[lo:lo + ne_loc]
        xc_c = xc[c * NC_LOC:(c + 1) * NC_LOC]     # [512, 512]
        cb_c = cb_s[c * NC_LOC:(c + 1) * NC_LOC]
        m = dict(common)
        m["ev"] = np.ascontiguousarray(
            xe_c.reshape(ET, P, NHID).transpose(1, 0, 2))
        m["evT"] = np.ascontiguousarray(
            xe_c.T.reshape(KT, P, ne_loc).transpose(1, 0, 2))
        m["clT"] = np.ascontiguousarray(
            xc_c.T.reshape(KT, P, NC_LOC).transpose(1, 0, 2))
        m["weh"] = (MAG * (eb_c[None, :] == g_ids[:PROJ, None])).astype(nbf16)
        m["wch"] = (MAG * (cb_c[None, :] == g_ids[:PROJ, None])).astype(nbf16)
        m["ohs"] = np.ascontiguousarray(
            (cb_c.reshape(CT, P)[:, :, None] == g_ids[None, None, :])
            .transpose(1, 0, 2)).astype(nbf16)
        in_maps.append(m)
    return in_maps, counts, ba, ne_loc


def postprocess(results: list, counts: np.ndarray, ba: np.ndarray) -> np.ndarray:
    seg = np.zeros((NG, NHID), np.float64)
    for c in range(N_CORES):
        seg += results[c]["seg"].astype(np.float64)
    # segment_mean(a + ba) = segment_mean(a) + ba, except empty graphs stay 0
    out = seg / np.maximum(counts, 1.0)[:, None] + (counts > 0)[:, None] * ba[None, :]
    return out.astype(np.float32)


def kernel(**inputs) -> np.ndarray:
    in_maps, counts, ba, ne_loc = make_in_maps(inputs)
    nc = build_nc(ne_loc=ne_loc)
    res = run_bass_kernel_spmd(nc, in_maps, list(range(N_CORES)))
    return postprocess(res.results, counts, ba)


# revision 15
# speedup vs baseline: 1.3295x; 1.0266x over previous
"""Trainium2 Bass kernel for the CrossAttention reference module.

  claim = x[claim_index]; evidence = x[evidence_index]
  wc = claim @ Wc + bc; we = evidence @ We + be
  S = wc @ we.T + blockdiag_mask(batch[claim_index], batch[evidence_index])
  A = softmax(S, -1); cn = A @ evidence
  a = concat([claim, cn, claim-cn, claim*cn]) @ Wa + ba
  out = segment_mean(a, batch[claim_index], 64)

Sharding (per the hint: devices hold gathered claim/evidence rows):
claims are sorted by graph and split 512 per core across 8 cores.
Evidence is sorted by graph on the host, so each core's evidence set is a
CONTIGUOUS slice of the sorted-evidence matrix (max span 1236 rows for
these inputs -> NE_LOC rounded to 1280).  The host pre-gathers x rows into
bf16 matrices tiled in the exact SBUF layouts the kernel wants (row-major
and transposed), so the device does plain contiguous DMAs: no gathers, no
on-device transposes, no dtype casts.

Device-side math (per core), all matmuls bf16 with fp32 PSUM accumulation:
  we_p^T = We^T @ evT + be            [64, NE_LOC]   (+ 64 one-hot rows from host)
  wc_p^T = Wc^T @ clT + bc            [64, 512]      (+ 64 one-hot rows)
  S^T[e-tile] = we_aug[:,e128]^T . wc_aug   -> [128e, 512c] per tile
  P^T = exp(S^T - 1074)               (one-hot trick adds +1024 on same-graph
                                       pairs; exp underflows to exactly 0 for
                                       cross-graph pairs, and the -50 shift is
                                       row-constant so softmax ignores it)
  cn^T[h] += ev[e,h128]^T . P^T       (PV with NO transposes: evidence rows are
  rowsum  += ones^T . P^T              the contraction dim in S^T space)
  cn^T *= 1/rowsum (broadcast via ones-matmul)
  a = clT^T.W1' + cnT^T.W2' + (clT*cnT)^T.W3'   (Wa folded on host:
        W1'=Wa0+Wa2, W2'=Wa1-Wa2, W3'=Wa3 -- the claim-cn concat block folds away)
  seg = onehot(graph)^T . a           (segment sums; host divides by counts
                                       and adds ba: mean(a+ba)=mean(a)+ba)
"""

import sys

if "/opt/trn_rl_repo" not in sys.path:
    sys.path.insert(0, "/opt/trn_rl_repo")

import ml_dtypes
import numpy as np

import concourse.bass as bass
import concourse.mybir as mybir
import concourse.tile as tile
from concourse.bass_utils import run_bass_kernel_spmd
from concourse.vector_clock import ScopedClock

P = 128
NHID = 512
PROJ = 64
NC_ALL = 4096
NE = 8192
NG = 64
N_CORES = 8
NC_LOC = NC_ALL // N_CORES  # 512 claims per core
KT = NHID // P              # 4 hidden k-tiles
CT = NC_LOC // P            # 4 claim tiles per core
KO = 12                     # folded concat k-tiles (3 blocks x 4)
MAG = 32.0                  # sqrt(1024): one-hot scale
EXP_BIAS = -(MAG * MAG + 50.0)  # exp(S + 1024 - 1074) = exp(S - 50)

f32 = mybir.dt.float32
bf16 = mybir.dt.bfloat16
AF = mybir.ActivationFunctionType
ALU = mybir.AluOpType
nbf16 = ml_dtypes.bfloat16


class _PatchedTileContext(tile.TileContext):
    """Workaround: this neuronxcc/walrus build rejects InstDrain carrying
    sync waits ("Too many sync wait commands").  Collect the final drain's
    waits on nops (one wait each) and emit the drain itself wait-free.
    Also slimmed teardown: one barrier instead of two (the second barrier
    only guarded re-execution racing the sem clear, which NRT's serialized
    executions already prevent)."""

    def _drain_and_barrier(self, tick_clock, wait_clock):
        nc = self.nc
        nop0 = nc.sync.nop(nofuse=True)
        wait_clock.add_sem_waits(nop0.ins, ScopedClock({None: tick_clock.global_clock}))
        si = nop0.ins.sync_info
        waits = list(si.on_wait) if si and si.on_wait else []
        if si and len(waits) > 1:
            del si.on_wait[1:]
            for w in waits[1:]:
                extra = nc.sync.nop(nofuse=True)
                if extra.ins.sync_info is None:
                    extra.ins.sync_info = mybir.SyncInfo(on_wait=[w], on_update=[])
                else:
                    extra.ins.sync_info.on_wait.append(w)
        drain_inst = nc.sync.drain()
        wait_clock.add_sem_waits(
            drain_inst.ins, ScopedClock({None: tick_clock.global_clock})
        )
        dsi = drain_inst.ins.sync_info
        if dsi and dsi.on_wait:
            del dsi.on_wait[:]
        nc.all_engine_barrier()
        popped = nc._tile_sem_poison_stack.pop()
        assert popped is self._sem_poison
        nc.clear_and_free_semaphores(list(self.sems.allocated().values()))


def _split_excess_waits(nc: bass.Bass, limit: int = 1) -> None:
    """This walrus build rejects instructions carrying more than ~1 sync
    wait.  Move excess waits onto injected same-engine nops (engines are
    in-order, so gating a preceding nop gates the instruction)."""
    for f in nc.m.functions:
        for bb in f.blocks:
            new_insts = []
            for inst in bb.instructions:
                si = getattr(inst, "sync_info", None)
                if si is not None and si.on_wait and len(si.on_wait) > limit:
                    keep = list(si.on_wait[-limit:])
                    excess = list(si.on_wait[:-limit])
                    for w in excess:
                        nop = mybir.InstNoOp(
                            name=f"I-{nc.next_id()}", engine=inst.engine,
                            ins=[], outs=[],
                            sync_info=mybir.SyncInfo(on_wait=[w], on_update=[]))
                        new_insts.append(nop)
                    del si.on_wait[:]
                    si.on_wait.extend(keep)
                new_insts.append(inst)
            bb.instructions[:] = new_insts


def build_nc(ne_loc: int = 1280, reps: int = 1) -> bass.Bass:
    assert ne_loc % P == 0
    ET = ne_loc // P
    # projection e-chunks (PSUM free dim is 512 f32 max)
    chunks = []
    off = 0
    while off < ne_loc:
        w = min(512, ne_loc - off)
        chunks.append((off, w))
        off += w

    nc = bass.Bass("TRN2", target_bir_lowering=False, debug=False,
                   num_devices=N_CORES)

    ev_d = nc.dram_tensor("ev", [P, ET, NHID], bf16, kind="ExternalInput").ap()
    evT_d = nc.dram_tensor("evT", [P, KT, ne_loc], bf16, kind="ExternalInput").ap()
    clT_d = nc.dram_tensor("clT", [P, KT, NC_LOC], bf16, kind="ExternalInput").ap()
    weh_d = nc.dram_tensor("weh", [PROJ, ne_loc], bf16, kind="ExternalInput").ap()
    wch_d = nc.dram_tensor("wch", [PROJ, NC_LOC], bf16, kind="ExternalInput").ap()
    wpb_d = nc.dram_tensor("wpb", [P, 2 * KT, PROJ], bf16, kind="ExternalInput").ap()
    wab_d = nc.dram_tensor("wab", [P, KO, NHID], bf16, kind="ExternalInput").ap()
    bce_d = nc.dram_tensor("bce", [PROJ, 2], f32, kind="ExternalInput").ap()
    ohs_d = nc.dram_tensor("ohs", [P, CT, NG], bf16, kind="ExternalInput").ap()
    seg_d = nc.dram_tensor("seg", [NG, NHID], f32, kind="ExternalOutput").ap()

    import os as _os
    N_WARM = int(_os.environ.get("KWARM", "7"))  # HAM warm-up matmul count

    with _PatchedTileContext(nc) as tc:
        with (
            tc.tile_pool(name="const", bufs=1) as cpool,
            # PSUM banks: s-ring 3 + pv 4 + rowsum 1 = 8 of 8
            tc.tile_pool(name="psS", bufs=3, space="PSUM") as psS,
            tc.tile_pool(name="psV", bufs=1, space="PSUM") as psV,
        ):
            # ---------- constants ----------
            # gpsimd memsets first (independent of DMA)
            scratch = cpool.tile([P, NHID], bf16)
            nc.gpsimd.memset(scratch[:], 0.5)
            exp_bias = cpool.tile([P, 1], f32)
            nc.gpsimd.memset(exp_bias[:], EXP_BIAS)
            ones_mat = cpool.tile([P, P], bf16)
            nc.gpsimd.memset(ones_mat[:], 1.0)
            # All loads on the sync HWDGE queue, in strict first-use order:
            # a single FIFO keeps DMA bandwidth on the critical-path tensor
            # instead of splitting it across concurrent queues.
            wpb_sb = cpool.tile([P, 2 * KT, PROJ], bf16)   # [Wc tiles | We tiles]
            nc.sync.dma_start(wpb_sb[:], wpb_d[:])
            wcb_sb = wpb_sb[:, :KT, :]
            web_sb = wpb_sb[:, KT:, :]
            clT_sb = cpool.tile([P, KT, NC_LOC], bf16)
            nc.sync.dma_start(clT_sb[:], clT_d[:])
            bce_sb = cpool.tile([PROJ, 2], f32)            # [bc | be]
            nc.sync.dma_start(bce_sb[:], bce_d[:])
            bc_sb = bce_sb[:, 0:1]
            be_sb = bce_sb[:, 1:2]
            evT_sb = cpool.tile([P, KT, ne_loc], bf16)
            for off, w in chunks:
                nc.sync.dma_start(evT_sb[:, :, off:off + w],
                                  evT_d[:, :, off:off + w])
            wc_aug = cpool.tile([P, NC_LOC], bf16)
            nc.sync.dma_start(wc_aug[PROJ:, :], wch_d[:])
            we_aug = cpool.tile([P, ne_loc], bf16)
            nc.sync.dma_start(we_aug[PROJ:, :], weh_d[:])
            ev_sb = cpool.tile([P, ET, NHID], bf16)
            ev_splits = [(0, 2)] + [(a, min(a + 4, ET)) for a in range(2, ET, 4)]
            for a, b in ev_splits:
                nc.sync.dma_start(ev_sb[:, a:b, :], ev_d[:, a:b, :])
            wab_sb = cpool.tile([P, KO, NHID], bf16)
            nc.sync.dma_start(wab_sb[:], wab_d[:])
            ohs_sb = cpool.tile([P, CT, NG], bf16)
            nc.sync.dma_start(ohs_sb[:], ohs_d[:])

            # ---------- HAM warm-up: garbage matmuls during DMA wait ----------
            # (into the s-ring; no readers, so the ring never stalls on them)
            for i in range(N_WARM):
                warm_ps = psS.tile([P, NHID], f32, tag="s")
                nc.tensor.matmul(warm_ps[:], scratch[:, :P], scratch[:],
                                 start=True, stop=True)

            with tc.tile_pool(name="work", bufs=2) as wpool:
                for rep in range(reps):
                    # ---------- projections (+ one-hot halves from host) ----------
                    ps = psS.tile([P, NHID], f32, tag="s")
                    for k in range(KT):
                        nc.tensor.matmul(ps[:PROJ, :], wcb_sb[:, k, :],
                                         clT_sb[:, k, :],
                                         start=(k == 0), stop=(k == KT - 1))
                    nc.scalar.activation(wc_aug[:PROJ, :], ps[:PROJ, :],
                                         AF.Identity, bias=bc_sb[:])
                    for off, w in chunks:
                        ps = psS.tile([P, NHID], f32, tag="s")
                        for k in range(KT):
                            nc.tensor.matmul(ps[:PROJ, :w], web_sb[:, k, :],
                                             evT_sb[:, k, off:off + w],
                                             start=(k == 0), stop=(k == KT - 1))
                        nc.scalar.activation(we_aug[:PROJ, off:off + w],
                                             ps[:PROJ, :w], AF.Identity,
                                             bias=be_sb[:])

                    # ---------- scores^T -> exp -> PV pipeline ----------
                    p_sb = wpool.tile([P, ET, NHID], bf16, tag="p")
                    pvs = [psV.tile([P, NC_LOC], f32, tag=f"pv{h}",
                                    name=f"pv{h}_{rep}") for h in range(KT)]
                    rs_ps = psV.tile([P, NC_LOC], f32, tag="rs",
                                     name=f"rs_{rep}")
                    for e in range(ET):
                        s_ps = psS.tile([P, NHID], f32, tag="s")
                        nc.tensor.matmul(s_ps[:], we_aug[:, e * P:(e + 1) * P],
                                         wc_aug[:], start=True, stop=True)
                        nc.scalar.activation(p_sb[:, e, :], s_ps[:], AF.Exp,
                                             bias=exp_bias[:])
                        # rowsum first (pre-broadcast to 128 partitions): its
                        # stop fires before the pv tail, so the ln/exp
                        # normalize chain overlaps the last pv matmuls
                        nc.tensor.matmul(rs_ps[:], ones_mat[:], p_sb[:, e, :],
                                         start=(e == 0), stop=(e == ET - 1))
                        for h in range(KT):
                            nc.tensor.matmul(pvs[h][:],
                                             ev_sb[:, e, h * P:(h + 1) * P],
                                             p_sb[:, e, :],
                                             start=(e == 0), stop=(e == ET - 1))

                    # ---------- normalize: 1/rs = exp(-ln(rs)) on ScalarE ----------
                    # (DVE reciprocal is an 8-cycle iterative op: 3.3us for 512
                    # elements; ACT ln+exp streams at 1 elem/cycle/lane and both
                    # live in the natural_log_exp_and_others table set)
                    lnr = wpool.tile([P, NC_LOC], f32, tag="lnr")
                    nc.scalar.activation(lnr[:], rs_ps[:], AF.Ln)
                    rbc = wpool.tile([P, NC_LOC], bf16, tag="rbc")
                    nc.scalar.activation(rbc[:], lnr[:], AF.Exp, scale=-1.0)

                    # ---------- aT blocks: cn^T and clT*cn^T ----------
                    # DVE streams the 4 cn normalizations; the elementwise
                    # products go to GpSimd (idle, runs in parallel) except
                    # the last which DVE picks up after cn3.
                    cnT = wpool.tile([P, KT, NC_LOC], bf16, tag="cnT")
                    mlT = wpool.tile([P, KT, NC_LOC], bf16, tag="mlT")
                    for h in range(KT):
                        nc.vector.tensor_tensor(out=cnT[:, h, :], in0=pvs[h][:],
                                                in1=rbc[:], op=ALU.mult)
                    for h in range(KT):
                        eng = nc.vector if h == KT - 1 else nc.gpsimd
                        eng.tensor_tensor(out=mlT[:, h, :],
                                          in0=clT_sb[:, h, :],
                                          in1=cnT[:, h, :], op=ALU.mult)

                    # ---------- a = aT^T @ Wa' ----------
                    a_out = wpool.tile([P, CT, NHID], bf16, tag="aout")
                    blocks = ([clT_sb[:, h, :] for h in range(KT)]
                              + [cnT[:, h, :] for h in range(KT)]
                              + [mlT[:, h, :] for h in range(KT)])
                    for t in range(CT):
                        o_ps = psS.tile([P, NHID], f32, tag="s")
                        for j, blk in enumerate(blocks):
                            nc.tensor.matmul(o_ps[:], blk[:, t * P:(t + 1) * P],
                                             wab_sb[:, j, :], start=(j == 0),
                                             stop=(j == KO - 1))
                        nc.scalar.copy(a_out[:, t, :], o_ps[:])

                    # ---------- segment sum via one-hot matmul ----------
                    seg_ps = psS.tile([P, NHID], f32, tag="s")
                    for t in range(CT):
                        nc.tensor.matmul(seg_ps[:NG, :], ohs_sb[:, t, :],
                                         a_out[:, t, :], start=(t == 0),
                                         stop=(t == CT - 1))
                    seg_sb = wpool.tile([NG, NHID], f32, tag="segsb")
                    nc.scalar.copy(seg_sb[:], seg_ps[:NG, :])
                    nc.sync.dma_start(seg_d[:], seg_sb[:])
    _split_excess_waits(nc)
    return nc


def make_in_maps(inputs: dict) -> tuple[list[dict], np.ndarray, np.ndarray, int]:
    """Host-side sharding: sort claims+evidence by graph, pre-gather x rows
    (bf16) into per-core contiguous slices tiled in SBUF layouts."""
    batch = np.asarray(inputs["batch"]).astype(np.int64)
    ci = np.asarray(inputs["claim_index"]).astype(np.int64)
    ei = np.asarray(inputs["evidence_index"]).astype(np.int64)
    x = np.asarray(inputs["x"], dtype=np.float32)
    cb = batch[ci]
    eb = batch[ei]
    counts = np.bincount(cb, minlength=NG).astype(np.float32)
    ba = np.asarray(inputs["ba"], dtype=np.float32).reshape(NHID)

    order_c = np.argsort(cb, kind="stable")
    cb_s = cb[order_c]
    order_e = np.argsort(eb, kind="stable")
    eb_s = eb[order_e]

    x_bf = x.astype(nbf16)
    xc = x_bf[ci[order_c]]          # [4096, 512] sorted claims
    xe = x_bf[ei[order_e]]          # [8192, 512] sorted evidence
    ev_starts = np.searchsorted(eb_s, np.arange(NG + 1))

    # per-core contiguous evidence spans
    spans = []
    for c in range(N_CORES):
        g_lo = int(cb_s[c * NC_LOC])
        g_hi = int(cb_s[(c + 1) * NC_LOC - 1])
        lo, hi = int(ev_starts[g_lo]), int(ev_starts[g_hi + 1])
        spans.append((lo, hi))
    ne_loc = max(512, -(-max(hi - lo for lo, hi in spans) // P) * P)
    ne_loc = min(ne_loc, NE)
    ET = ne_loc // P

    Wc = np.asarray(inputs["Wc"], dtype=np.float32)
    We = np.asarray(inputs["We"], dtype=np.float32)
    Wa = np.asarray(inputs["Wa"], dtype=np.float32)
    W1 = Wa[0:NHID] + Wa[2 * NHID:3 * NHID]
    W2 = Wa[NHID:2 * NHID] - Wa[2 * NHID:3 * NHID]
    W3 = Wa[3 * NHID:4 * NHID]
    wab = np.concatenate([W1, W2, W3], axis=0).astype(nbf16)  # [1536, 512]

    def tile_kpm(w, k):  # [(k p), m] -> [p, k, m]
        return np.ascontiguousarray(
            w.reshape(k, P, -1).transpose(1, 0, 2))

    g_ids = np.arange(NG)
    common = {
        "wpb": np.concatenate([tile_kpm(Wc.astype(nbf16), KT),
                               tile_kpm(We.astype(nbf16), KT)], axis=1),
        "wab": tile_kpm(wab, KO),
        "bce": np.stack([np.asarray(inputs["bc"], dtype=np.float32).reshape(PROJ),
                         np.asarray(inputs["be"], dtype=np.float32).reshape(PROJ)],
                        axis=1),
    }
    in_maps = []
    for c in range(N_CORES):
        lo, hi = spans[c]
        lo = min(lo, NE - ne_loc)
        xe_c = xe[lo:lo + ne_loc]                  # [ne_loc, 512]
        eb_c = eb_s[lo:lo + ne_loc]
        xc_c = xc[c * NC_LOC:(c + 1) * NC_LOC]     # [512, 512]
        cb_c = cb_s[c * NC_LOC:(c + 1) * NC_LOC]
        m = dict(common)
        m["ev"] = np.ascontiguousarray(
            xe_c.reshape(ET, P, NHID).transpose(1, 0, 2))
        m["evT"] = np.ascontiguousarray(
            xe_c.T.reshape(KT, P, ne_loc).transpose(1, 0, 2))
        m["clT"] = np.ascontiguousarray(
            xc_c.T.reshape(KT, P, NC_LOC).transpose(1, 0, 2))
        m["weh"] = (MAG * (eb_c[None, :] == g_ids[:PROJ, None])).astype(nbf16)
        m["wch"] = (MAG * (cb_c[None, :] == g_ids[:PROJ, None])).astype(nbf16)
        m["ohs"] = np.ascontiguousarray(
            (cb_c.reshape(CT, P)[:, :, None] == g_ids[None, None, :])
            .transpose(1, 0, 2)).astype(nbf16)
        in_maps.append(m)
    return in_maps, counts, ba, ne_loc


def postprocess(results: list, counts: np.ndarray, ba: np.ndarray) -> np.ndarray:
    seg = np.zeros((NG, NHID), np.float64)
    for c in range(N_CORES):
        seg += results[c]["seg"].astype(np.float64)
    # segment_mean(a + ba) = segment_mean(a) + ba, except empty graphs stay 0
    out = seg / np.maximum(counts, 1.0)[:, None] + (counts > 0)[:, None] * ba[None, :]
    return out.astype(np.float32)


def kernel(**inputs) -> np.ndarray:
    in_maps, counts, ba, ne_loc = make_in_maps(inputs)
    nc = build_nc(ne_loc=ne_loc)
    res = run_bass_kernel_spmd(nc, in_maps, list(range(N_CORES)))
    return postprocess(res.results, counts, ba)
